# revision 1
# baseline (speedup 1.0000x reference)
"""Trainium2 Bass kernel for PointConv message passing (e3nn UVU tensor product).

Self-contained: accepts FULL inputs, shards edges across 8 NeuronCores,
runs one SPMD Bass program, returns the FULL [E, 128] message tensor.

Sharding: edges bucketed by source node; core c owns nodes [c*npc,(c+1)*npc).
Host greedily packs consecutive nodes into subwindows (<=64 nodes, <=1024
edge slots, ~99% slot utilization) and remaps the node table so subwindow s
occupies node rows [64s, 64s+64) — keeping all device-side addressing
SPMD-uniform. A macro = 2 subwindows = 2048 edge slots (16 j-blocks of 128).

Per-core pipeline (edge tensors bf16; 1o irreps i-major):
  Phase Y: y = linear_1(nf) in bf16 from a host-pretransposed node table
    (no PE transposes), kept SBUF-resident in per-4-macro chunk tiles that
    the gathers read in place (base-0/64 partition slices); emitted
    interleaved with edge macros so it fully overlaps.
  Per macro:
    PE:  mlp1 (host-pretransposed emb), mlp2 (block-diag, strided PSUM
         outs) -> per-edge TP weights [w0|w2|w1|w3]; one-hot gathers
         (fp8 x bf16) -> X.
    ACT: silu + PSUM->SBUF bf16 copies (W, X halves).
    DVE: 9 bf16 tensor ops, all in 2x mode (broadcasts only on middle AP
         dims; A1REP = a1 replicated over channels arrives via DMA from a
         host-materialized table).
  Output [E,128] bf16; host converts to f32 and un-permutes i-major cols.
"""

import dataclasses
import sys
import types

sys.path.insert(0, "/opt/trn_rl_repo")


def _install_axon_hooks():
    """The image's antenv package lacks axon_hooks (NTFF profiling hook
    storage); inject an equivalent so trace=True works under axon."""
    if "antenv.axon_hooks" in sys.modules:
        return
    state = {"hook": None, "tried": False}
    mod = types.ModuleType("antenv.axon_hooks")

    def set_axon_ntff_profile_hook(h):
        state["hook"] = h
        state["tried"] = True

    def get_axon_ntff_profile_hook():
        if state["hook"] is None and not state["tried"]:
            state["tried"] = True
            try:
                from trn_agent_boot.trn_boot import _ntff_profile_via_ctypes

                state["hook"] = _ntff_profile_via_ctypes(
                    "/opt/axon/libaxon_pjrt.so"
                )
            except Exception:
                state["hook"] = None
        return state["hook"]

    mod.set_axon_ntff_profile_hook = set_axon_ntff_profile_hook
    mod.get_axon_ntff_profile_hook = get_axon_ntff_profile_hook
    sys.modules["antenv.axon_hooks"] = mod
    try:
        import antenv

        antenv.axon_hooks = mod
    except Exception:
        pass


_install_axon_hooks()

import numpy as np  # noqa: E402
import ml_dtypes  # noqa: E402
import concourse.bass as bass  # noqa: E402,F401
import concourse.bacc as bacc  # noqa: E402
import concourse.tile as tile  # noqa: E402
import concourse.mybir as mybir  # noqa: E402
from concourse import bass_utils  # noqa: E402

bass_utils.upload_artifacts = lambda tmpdir: f"file://{tmpdir}"

F32 = mybir.dt.float32
BF16 = mybir.dt.bfloat16
FP8 = mybir.dt.float8e4
AOP = mybir.AluOpType
AFT = mybir.ActivationFunctionType
AXL = mybir.AxisListType
BF16NP = ml_dtypes.bfloat16
FP8NP = ml_dtypes.float8_e4m3

P = 128
MULC = 32  # irrep multiplicity
EMBD = 16
HID = 8
J = 16  # 128-edge sub-blocks per macro tile
B = P * J  # 2048 edge slots per macro tile
SWN = 64  # max nodes per subwindow (y rows per subwindow)
YRM = 128  # y rows per macro (two subwindows)
BS = 1024  # edge slots per subwindow
N_CORES = 8


def _fd(view, off, dims):
    """Replace the free dims of an AP with custom (step, count) pairs."""
    return dataclasses.replace(
        view,
        offset=view.offset + off,
        ap=[list(view.ap[0])] + [[s, c] for s, c in dims],
    )


def _mk(view, off, dims):
    """Replace the whole AP (all dims) with custom (step, count) pairs."""
    return dataclasses.replace(
        view,
        offset=view.offset + off,
        ap=[[s, c] for s, c in dims],
    )


def build_program(nm, n_cores=N_CORES):
    """nm: macros per core. y table rows chunked 512 per 4 macros."""
    NG = -(-nm // 4)
    npad = NG * 4 * P
    NE = nm * B  # edge slots per core

    nc = bacc.Bacc(
        "TRN2",
        target_bir_lowering=False,
        debug=False,
        enable_asserts=False,
        num_devices=n_cores,
    )
    nf = nc.dram_tensor("nf", [npad, P], BF16, kind="ExternalInput").ap()
    embT = nc.dram_tensor("embT", [nm * P, 2 * P], BF16, kind="ExternalInput").ap()
    a1r = nc.dram_tensor("a1r", [nm * P, J * 96], BF16, kind="ExternalInput").ap()
    sab = nc.dram_tensor("sab", [nm * P, BS], FP8, kind="ExternalInput").ap()
    wlin = nc.dram_tensor("wlin", [P, P], BF16, kind="ExternalInput").ap()
    w1b = nc.dram_tensor("w1b", [P, 64], BF16, kind="ExternalInput").ap()
    w2r = nc.dram_tensor("w2r", [P, 1024], BF16, kind="ExternalInput").ap()
    msg = nc.dram_tensor("msg", [NE, P], BF16, kind="ExternalOutput").ap()

    with tile.TileContext(nc) as tc:
        with (
            tc.tile_pool(name="consts", bufs=1) as cpool,
            tc.tile_pool(name="dram", bufs=1, space="DRAM") as dpool,
            tc.tile_pool(name="sbi", bufs=4) as sbi,
            tc.tile_pool(name="sbw", bufs=4) as sbw,
            tc.tile_pool(name="ps", bufs=2, space="PSUM") as ps,
        ):
            WLIN = cpool.tile([P, P], BF16)
            nc.sync.dma_start(out=WLIN[:], in_=wlin)
            W1B = cpool.tile([P, 64], BF16)
            nc.sync.dma_start(out=W1B[:], in_=w1b)
            W2R = cpool.tile([P, 1024], BF16)
            nc.sync.dma_start(out=W2R[:], in_=w2r)
            ys_tiles = {}

            # ---- phase Y: y = linear_1(node_feats), bf16 i-major table,
            # emitted interleaved with edge macros (2 chunks lookahead) ----
            def emit_y_group(g):
                g0 = 4 * g
                gn = 4
                w = gn * P
                nfT = sbi.tile([P, 4 * P], BF16, tag="nfT")
                nc.sync.dma_start(
                    out=nfT[:, :w],
                    in_=_mk(nf, g0 * P * P, [(P, P), (P * P, gn), (1, P)]),
                )
                yp = ps.tile([P, 4 * P], F32, tag="w", bufs=2)
                for t in range(gn):
                    nc.tensor.matmul(
                        out=yp[:, t * P : (t + 1) * P],
                        lhsT=nfT[:, t * P : (t + 1) * P],
                        rhs=WLIN[:],
                        start=True,
                        stop=True,
                    )
                ys = sbi.tile([P, 4 * P], BF16, tag="ys")
                nc.scalar.copy(out=ys[:, :w], in_=yp[:, :w])
                ys_tiles[g] = ys

            emit_y_group(0)

            # ---- edge phase ----
            for m in range(nm):
                if m == 0 and NG > 1:
                    emit_y_group(1)
                if m % 4 == 1 and m // 4 + 2 < NG:
                    emit_y_group(m // 4 + 2)
                e0 = m * B
                ETS = sbi.tile([P, 2 * P], BF16, tag="ets")
                nc.sync.dma_start(out=ETS[:], in_=embT[m * P : (m + 1) * P, :])
                A1R = sbi.tile([P, J * 96], BF16, tag="a1r")
                nc.sync.dma_start(out=A1R[:], in_=a1r[m * P : (m + 1) * P, :])
                SAB = sbi.tile([P, BS], FP8, tag="sab")
                nc.sync.dma_start(out=SAB[:], in_=sab[m * P : (m + 1) * P, :])
                ysg = ys_tiles[m // 4]
                yt0 = (m % 4) * P

                # mlp1: h = emb @ w1 (block-diag over 8 jj), transposed layout
                hpx = ps.tile([P, 1024], F32, tag="x", bufs=2)
                nc.tensor.matmul(
                    out=hpx[0:64, 0:P], lhsT=W1B[:], rhs=ETS[:, 0:P],
                    start=True, stop=True,
                )
                nc.tensor.matmul(
                    out=hpx[64:P, 0:P], lhsT=W1B[:], rhs=ETS[:, P : 2 * P],
                    start=True, stop=True,
                )
                HSM = sbw.tile([P, P], BF16, tag="hsm")
                nc.scalar.activation(
                    out=HSM[:], in_=hpx[:, 0:P], func=AFT.Silu
                )

                # mlp2 + a1rep -> W tile; per j: [w0|w2|w1|w3|a1rep(96)|pad]
                WS = sbw.tile([P, J * P], BF16, tag="ws")
                whs = []
                for t in range(2):
                    wh = ps.tile([P, 1024], F32, tag="w", bufs=2)
                    for jp in range(4):
                        nc.tensor.matmul(
                            out=wh[:, 256 * jp : 256 * (jp + 1)],
                            lhsT=HSM[64 * t : 64 * (t + 1), :],
                            rhs=W2R[64 * t : 64 * (t + 1), 256 * jp : 256 * (jp + 1)],
                            start=True,
                            stop=True,
                        )
                    whs.append(wh)

                # one-hot gathers -> X  (per j: [x0 | x1 i-major])
                XS = sbw.tile([P, B], BF16, tag="xs")
                for h in range(2):
                    xp = ps.tile([P, 1024], F32, tag="x", bufs=2)
                    for jj in range(8):
                        nc.tensor.matmul(
                            out=xp[:, P * jj : P * (jj + 1)],
                            lhsT=SAB[64 * h : 64 * (h + 1), P * jj : P * (jj + 1)],
                            rhs=ysg[64 * h : 64 * (h + 1), yt0 : yt0 + P],
                            start=True,
                            stop=True,
                        )
                    nc.scalar.copy(
                        out=XS[:, 1024 * h : 1024 * (h + 1)], in_=xp[:]
                    )
                for t in range(2):
                    nc.scalar.copy(
                        out=WS[:, 1024 * t : 1024 * (t + 1)], in_=whs[t]
                    )

                # ---- tensor product (bf16, 2x mode) ----
                M1 = sbw.tile([P, J * 96], BF16, tag="m1")
                DD = sbw.tile([P, J * 32], BF16, tag="dd")
                T02 = sbw.tile([P, J * 64], BF16, tag="t02")
                TD = sbw.tile([P, J * 96], BF16, tag="td")
                OUT = sbw.tile([P, B], BF16, tag="out")

                WSv, XSv = WS[:], XS[:]
                w02 = _fd(WSv, 0, [(P, J), (32, 2), (1, 32)])
                w1v = _fd(WSv, 64, [(P, J), (1, 32)])
                w3v = _fd(WSv, 96, [(P, J), (0, 3), (1, 32)])
                a1f = _fd(A1R[:], 0, [(96, J), (1, 96)])
                a1s = _fd(A1R[:], 0, [(96, J), (32, 3), (1, 32)])
                x0r2 = _fd(XSv, 0, [(P, J), (0, 2), (1, 32)])
                x1f = _fd(XSv, 32, [(P, J), (1, 96)])
                x1s = _fd(XSv, 32, [(P, J), (32, 3), (1, 32)])
                m1f = _fd(M1[:], 0, [(96, J), (1, 96)])
                m1a = _fd(M1[:], 0, [(96, J), (1, 32)])
                m1b = _fd(M1[:], 32, [(96, J), (1, 32)])
                m1c = _fd(M1[:], 64, [(96, J), (1, 32)])
                ddv = _fd(DD[:], 0, [(32, J), (1, 32)])
                t02o = _fd(T02[:], 0, [(64, J), (32, 2), (1, 32)])
                t0v = _fd(T02[:], 0, [(64, J), (1, 32)])
                t2b = _fd(T02[:], 32, [(64, J), (0, 3), (1, 32)])
                tdf = _fd(TD[:], 0, [(96, J), (1, 96)])
                tds = _fd(TD[:], 0, [(96, J), (32, 3), (1, 32)])
                out0v = _fd(OUT[:], 0, [(P, J), (1, 32)])
                out1s = _fd(OUT[:], 32, [(P, J), (32, 3), (1, 32)])
                TT = nc.vector.tensor_tensor

                TT(out=m1f, in0=x1f, in1=a1f, op=AOP.mult)
                TT(out=ddv, in0=m1a, in1=m1b, op=AOP.add)
                TT(out=ddv, in0=ddv, in1=m1c, op=AOP.add)
                TT(out=ddv, in0=w1v, in1=ddv, op=AOP.mult)
                TT(out=t02o, in0=w02, in1=x0r2, op=AOP.mult)
                TT(out=tds, in0=w3v, in1=x1s, op=AOP.mult)
                TT(out=out1s, in0=t2b, in1=a1s, op=AOP.mult)
                TT(out=out1s, in0=out1s, in1=tds, op=AOP.add)
                TT(out=out0v, in0=t0v, in1=ddv, op=AOP.add)

                nc.sync.dma_start(
                    out=msg[e0 : e0 + B, :].rearrange("(p j) q -> p (j q)", p=P),
                    in_=OUT[:],
                )

    nc.compile()
    return nc


def make_consts(lin_w0, lin_w1, mlp_w1, mlp_w2):
    S3 = 3.0 ** -0.5
    S2 = 2.0 ** -0.5
    sl = MULC ** -0.5
    # linear_1 with i-major y layout: y col 32+32i+v <- x row 32+3u+i
    wlin = np.zeros((P, P), np.float32)
    wlin[:MULC, :MULC] = lin_w0 * sl
    u_arr = np.arange(MULC)
    for i in range(3):
        wlin[np.ix_(MULC + 3 * u_arr + i, MULC + MULC * i + u_arr)] = (
            lin_w1 * sl
        )
    # mlp1 block-diag (8 subwindow-j blocks of [16, 8])
    w1b = np.zeros((P, 64), np.float32)
    w1s = (mlp_w1 * EMBD ** -0.5).astype(np.float32)
    for jj in range(8):
        w1b[EMBD * jj : EMBD * (jj + 1), HID * jj : HID * (jj + 1)] = w1s
    # mlp2, col order [w0|w2|w1|w3], path scales folded; block-diag j-pair
    w2s = (mlp_w2 * HID ** -0.5).astype(np.float32)
    w2c = np.concatenate(
        [
            S2 * w2s[:, 0:32],
            S2 * S3 * w2s[:, 64:96],
            S2 * S3 * w2s[:, 32:64],
            S2 * S3 * w2s[:, 96:128],
        ],
        axis=1,
    )  # [8, 128]
    w2r = np.zeros((128, 1024), np.float32)
    for jj in range(8):
        w2r[jj * 8 : (jj + 1) * 8, jj * 128 : (jj + 1) * 128] = w2c
    w2r[64:128] = w2r[0:64]
    return (
        wlin.astype(BF16NP),
        w1b.astype(BF16NP),
        w2r.astype(BF16NP),
    )


_PROGRAM_CACHE = {}


def _get_program(nm):
    if nm not in _PROGRAM_CACHE:
        _PROGRAM_CACHE[nm] = build_program(nm)
    return _PROGRAM_CACHE[nm]


def kernel(
    node_feats,
    edge_attrs,
    edge_embedding,
    edge_src,
    edge_dst,
    lin_w0,
    lin_w1,
    mlp_w1,
    mlp_w2,
):
    node_feats = np.ascontiguousarray(np.asarray(node_feats, np.float32))
    edge_attrs = np.ascontiguousarray(np.asarray(edge_attrs, np.float32))
    edge_embedding = np.ascontiguousarray(np.asarray(edge_embedding, np.float32))
    edge_src = np.asarray(edge_src, np.int64)
    lin_w0 = np.asarray(lin_w0, np.float32)
    lin_w1 = np.asarray(lin_w1, np.float32)
    mlp_w1 = np.asarray(mlp_w1, np.float32)
    mlp_w2 = np.asarray(mlp_w2, np.float32)

    E = edge_src.shape[0]
    N = node_feats.shape[0]
    npc = -(-N // N_CORES)  # nodes per core

    # greedy-pack each core's nodes into subwindows (<=SWN nodes, <=BS edges)
    deg = np.bincount(edge_src, minlength=N)
    swid_g = np.empty(N, np.int64)  # subwindow id within core
    koff_g = np.empty(N, np.int64)  # node row within subwindow
    nsw = np.zeros(N_CORES, np.int64)
    for c in range(N_CORES):
        lo, hi = c * npc, min(N, (c + 1) * npc)
        s, edges, nodes = 0, 0, 0
        for n in range(lo, hi):
            d = deg[n]
            if edges + d > BS or nodes == SWN:
                s += 1
                edges, nodes = 0, 0
            swid_g[n] = s
            koff_g[n] = nodes
            edges += d
            nodes += 1
        nsw[c] = s + 1
    nm = int(-(-nsw.max() // 2))
    npad = -(-nm // 4) * 4 * P
    NE = nm * B

    perm = np.argsort(edge_src, kind="stable")
    src_s = edge_src[perm]
    core_of = src_s // npc
    sw_s = swid_g[src_s]
    m_of = sw_s // 2
    sub_of = sw_s % 2
    kloc = koff_g[src_s]

    # slot index within each (core, subwindow) group (groups are contiguous)
    grp = core_of * (2 * nm) + sw_s
    cnt = np.bincount(grp, minlength=N_CORES * nm * 2)
    assert cnt.max() <= BS
    gstart = np.concatenate([[0], np.cumsum(cnt)])[:-1]
    ssub = np.arange(E) - gstart[grp]
    p_ = ssub // 8
    jj8 = ssub % 8
    j_ = 8 * sub_of + jj8
    gslot = m_of * B + 16 * p_ + j_  # slot within the core's edge array

    wlin, w1b, w2r = make_consts(lin_w0, lin_w1, mlp_w1, mlp_w2)

    in_maps = []
    for c in range(N_CORES):
        sel = core_of == c
        ids = perm[sel]
        nf_c = np.zeros((npad, P), BF16NP)
        lo = c * npc
        hi = min(N, lo + npc)
        if hi > lo:
            rows = swid_g[lo:hi] * SWN + koff_g[lo:hi]
            nf_c[rows] = node_feats[lo:hi]
        nf_c = np.ascontiguousarray(
            nf_c.reshape(npad // P, P, P).transpose(0, 2, 1).reshape(npad, P)
        )
        emb_c = np.zeros((NE, EMBD), np.float32)
        att_c = np.zeros((NE, 4), np.float32)
        sab_c = np.zeros((nm * P, BS), FP8NP)
        if ids.shape[0]:
            gs = gslot[sel]
            emb_c[gs] = edge_embedding[ids]
            att_c[gs] = edge_attrs[ids]
            rows = m_of[sel] * P + SWN * sub_of[sel] + kloc[sel]
            cols = jj8[sel] * P + p_[sel]
            sab_c[rows, cols] = 1
        # embT: [nm, jj8*16+k, sub*128+p] <- emb[slot(16p+8sub+jj8), k]
        e4 = emb_c.reshape(nm, P, 2, 8, EMBD)  # [m, p, sub, jj8, k]
        embT_c = np.ascontiguousarray(
            e4.transpose(0, 3, 4, 2, 1).reshape(nm * P, 2 * P)
        ).astype(BF16NP)
        # a1r: [m*128+p, j*96+32i+u] <- att[slot(16p+j), 1+i]  (bcast over u)
        a4 = att_c.reshape(nm, P, J, 4)  # [m, p, j, kk]
        a1r_c = np.ascontiguousarray(
            np.broadcast_to(
                a4[:, :, :, 1:4, None], (nm, P, J, 3, MULC)
            ).reshape(nm * P, J * 96)
        ).astype(BF16NP)
        in_maps.append(
            {
                "nf": nf_c,
                "embT": embT_c,
                "a1r": a1r_c,
                "sab": sab_c,
                "wlin": wlin,
                "w1b": w1b,
                "w2r": w2r,
            }
        )

    nc = _get_program(nm)
    global _LAST_IN_MAPS
    _LAST_IN_MAPS = in_maps
    res = bass_utils.run_bass_kernel_spmd(
        nc, in_maps, core_ids=list(range(N_CORES))
    )
    # un-permute i-major 1o cols back to reference (u-major) order
    colperm = np.empty(P, np.int64)
    colperm[:MULC] = np.arange(MULC)
    for i in range(3):
        colperm[MULC + 3 * np.arange(MULC) + i] = MULC + MULC * i + np.arange(MULC)
    out = np.empty((E, P), np.float32)
    for c in range(N_CORES):
        sel = core_of == c
        ids = perm[sel]
        if ids.shape[0]:
            msg_c = np.asarray(res.results[c]["msg"], dtype=np.float32)
            out[ids] = msg_c[gslot[sel]][:, colperm]
    return out

